# revision 8
# baseline (speedup 1.0000x reference)
"""Gemma decoder layer on 8 Trainium2 NeuronCores.

Strategy: token-parallel (data-parallel over tokens), zero collectives.
Each core owns a 512-token strip (batch c//4, positions (c%4)*512..+512).
All weights replicated (bf16); every core computes full K/V for its batch
(MQA => tiny). All activations live in transposed layout [hidden->partitions,
tokens->free]; RMSNorm partition-reduction via ones-matmul; norm weights
(1+w) folded into the following matmul weights on the host. The program is
identical across cores (SPMD) - all per-core variation enters as input data.

Matmuls in bf16 (fp32 PSUM accumulation); softmax/norm/rope/residual in fp32.
"""

from contextlib import ExitStack

import numpy as np
import ml_dtypes

import concourse.bass as bass
import concourse.mybir as mybir
import concourse.tile as tile
from concourse import bacc
from concourse.bass_utils import run_bass_kernel_spmd
from concourse.masks import make_identity

P = 128
B, S, HID = 2, 2048, 2048
NH, D = 8, 256
INTER = 16384
KC = HID // P          # 16 hidden chunks
TOWN = 512             # own tokens per core
TKV = 2048             # tokens per batch (keys)
EPS = 1e-6
SCALING = D ** -0.5
F32 = mybir.dt.float32
BF16 = mybir.dt.bfloat16
AF = mybir.ActivationFunctionType
ALU = mybir.AluOpType

TRACE = False
LAST_RESULTS = None
_NC_CACHE = None


def _build_nc():
    nc = bacc.Bacc(None, target_bir_lowering=False)

    # ---- DRAM I/O (shapes match SBUF layouts exactly; host does all packing) ----
    d_hskv = nc.dram_tensor("hskv", [P, KC, TKV], F32, kind="ExternalInput")
    d_hsown = nc.dram_tensor("hsown", [P, KC, TOWN], F32, kind="ExternalInput")
    d_coskv = nc.dram_tensor("coskv", [P, TKV], BF16, kind="ExternalInput")
    d_sinkv = nc.dram_tensor("sinkv", [P, TKV], BF16, kind="ExternalInput")
    d_cosown = nc.dram_tensor("cosown", [P, TOWN], BF16, kind="ExternalInput")
    d_sinown = nc.dram_tensor("sinown", [P, TOWN], BF16, kind="ExternalInput")
    d_mask = nc.dram_tensor("mask", [P, 4, TKV], BF16, kind="ExternalInput")
    d_qw = nc.dram_tensor("qw", [P, NH, KC, 2 * P], BF16, kind="ExternalInput")
    d_kw = nc.dram_tensor("kw", [P, 2, KC, P], BF16, kind="ExternalInput")
    d_vw = nc.dram_tensor("vw", [P, KC, D], BF16, kind="ExternalInput")
    d_ow = nc.dram_tensor("ow", [P, KC // 2, KC, 2 * P], BF16, kind="ExternalInput")
    d_gw = nc.dram_tensor("gw", [P, INTER // (2 * P), KC, 2 * P], BF16, kind="ExternalInput")
    d_uw = nc.dram_tensor("uw", [P, INTER // (2 * P), KC, 2 * P], BF16, kind="ExternalInput")
    d_dw = nc.dram_tensor("dw", [P, KC, INTER // P, P], BF16, kind="ExternalInput")
    d_out = nc.dram_tensor("out", [KC, P, TOWN], F32, kind="ExternalOutput")

    with tile.TileContext(nc) as tc, ExitStack() as top:
        const = top.enter_context(tc.tile_pool(name="const", bufs=1))
        ones_b = const.tile([P, P], BF16)
        nc.vector.memset(ones_b[:], 1.0)
        eps_sb = const.tile([P, 1], F32)
        nc.vector.memset(eps_sb[:], EPS)
        ident_b = const.tile([P, P], BF16)
        make_identity(nc, ident_b)
        cos_kv = const.tile([P, TKV], BF16)
        sin_kv = const.tile([P, TKV], BF16)
        cos_own = const.tile([P, TOWN], BF16)
        sin_own = const.tile([P, TOWN], BF16)
        nc.sync.dma_start(cos_kv[:], d_coskv[:])
        nc.sync.dma_start(sin_kv[:], d_sinkv[:])
        nc.sync.dma_start(cos_own[:], d_cosown[:])
        nc.sync.dma_start(sin_own[:], d_sinown[:])
        kw_sb = const.tile([P, 2, KC, P], BF16)
        vw_sb = const.tile([P, KC, D], BF16)
        nc.sync.dma_start(kw_sb[:], d_kw[:])
        nc.sync.dma_start(vw_sb[:], d_vw[:])

        def rms_norm(sb, ps, src, dst, T, tagp):
            """src [P, KC, T] f32 -> dst [P, KC, T] bf16 (no (1+w): folded in weights)"""
            ps_ss = ps.tile([P, T], F32, tag=f"ss{tagp}")
            for k in range(KC):
                x2 = sb.tile([P, T], BF16, tag=f"x2{tagp}")
                nc.scalar.activation(x2[:], src[:, k], AF.Square)
                nc.tensor.matmul(ps_ss[:], ones_b[:], x2[:],
                                 start=(k == 0), stop=(k == KC - 1))
            sd = sb.tile([P, T], F32, tag=f"sd{tagp}")
            nc.scalar.activation(sd[:], ps_ss[:], AF.Sqrt, scale=1.0 / HID,
                                 bias=eps_sb[:])
            rs = sb.tile([P, T], F32, tag=f"rs{tagp}")
            nc.vector.reciprocal(rs[:], sd[:])
            for k in range(KC):
                nc.vector.tensor_mul(dst[:, k], src[:, k], rs[:])

        # pools that span multiple phases (entered/exited manually, LIFO per side)
        p_attn_cm = tc.tile_pool(name="p_attn", bufs=1)  # P1..P4 (attn tile, used P3-P4)
        p_attn = p_attn_cm.__enter__()
        p13_cm = tc.tile_pool(name="p13", bufs=1)        # P1..P3: kTr, v_sb, qTr
        p13 = p13_cm.__enter__()
        kTr = p13.tile([P, 2, TKV], BF16)
        v_sb = p13.tile([P, TKV // P, D], BF16)

        # ============ P1: per-batch K/V (norm + proj + rope for all TKV tokens) ====
        ST = 256  # strip size
        with tc.tile_pool(name="s1", bufs=2) as s1, \
             tc.tile_pool(name="ps1", bufs=2, space="PSUM") as ps1:
            for s in range(TKV // ST):
                hsT = s1.tile([P, KC, ST], F32, tag="hsT")
                nc.sync.dma_start(hsT[:], d_hskv[:, :, s * ST:(s + 1) * ST])
                hb = s1.tile([P, KC, ST], BF16, tag="hb")
                rms_norm(s1, ps1, hsT, hb, ST, "kv")
                # K projection (2 feature tiles: x1, x2 halves of the single KV head)
                ps_k0 = ps1.tile([P, ST], F32, tag="pk0")
                ps_k1 = ps1.tile([P, ST], F32, tag="pk1")
                for k in range(KC):
                    nc.tensor.matmul(ps_k0[:], kw_sb[:, 0, k], hb[:, k],
                                     start=(k == 0), stop=(k == KC - 1))
                for k in range(KC):
                    nc.tensor.matmul(ps_k1[:], kw_sb[:, 1, k], hb[:, k],
                                     start=(k == 0), stop=(k == KC - 1))
                cs = cos_kv[:, s * ST:(s + 1) * ST]
                sn = sin_kv[:, s * ST:(s + 1) * ST]
                t1 = s1.tile([P, ST], F32, tag="rt1")
                t2 = s1.tile([P, ST], F32, tag="rt2")
                nc.vector.tensor_mul(t1[:], ps_k0[:], cs)
                nc.vector.tensor_mul(t2[:], ps_k1[:], sn)
                nc.vector.tensor_sub(kTr[:, 0, s * ST:(s + 1) * ST], t1[:], t2[:])
                t3 = s1.tile([P, ST], F32, tag="rt3")
                t4 = s1.tile([P, ST], F32, tag="rt4")
                nc.vector.tensor_mul(t3[:], ps_k0[:], sn)
                nc.vector.tensor_mul(t4[:], ps_k1[:], cs)
                nc.vector.tensor_add(kTr[:, 1, s * ST:(s + 1) * ST], t3[:], t4[:])
                # V projection, natural layout [token-part, D]
                for tt in range(ST // P):
                    ps_v = ps1.tile([P, D], F32, tag="pv")
                    for k in range(KC):
                        nc.tensor.matmul(ps_v[:], hb[:, k, tt * P:(tt + 1) * P],
                                         vw_sb[:, k], start=(k == 0), stop=(k == KC - 1))
                    nc.vector.tensor_copy(v_sb[:, s * (ST // P) + tt], ps_v[:])

        # ============ P2: own strip norm + Q proj + rope ============
        qTr = p13.tile([P, KC, TOWN], BF16)
        with tc.tile_pool(name="s2a", bufs=1) as s2a, \
             tc.tile_pool(name="s2", bufs=2) as s2, \
             tc.tile_pool(name="ps2", bufs=2, space="PSUM") as ps2:
            hs_own = s2a.tile([P, KC, TOWN], F32)
            nc.sync.dma_start(hs_own[:], d_hsown[:])
            hb_own = s2a.tile([P, KC, TOWN], BF16)
            rms_norm(s2, ps2, hs_own, hb_own, TOWN, "own")
            for h in range(NH):
                qw_t = s2.tile([P, KC, 2 * P], BF16, tag="qw")
                nc.sync.dma_start(qw_t[:], d_qw[:, h])
                ps_q0 = ps2.tile([P, TOWN], F32, tag="pq0")
                ps_q1 = ps2.tile([P, TOWN], F32, tag="pq1")
                for k in range(KC):
                    nc.tensor.matmul(ps_q0[:], qw_t[:, k, 0:P], hb_own[:, k],
                                     start=(k == 0), stop=(k == KC - 1))
                for k in range(KC):
                    nc.tensor.matmul(ps_q1[:], qw_t[:, k, P:2 * P], hb_own[:, k],
                                     start=(k == 0), stop=(k == KC - 1))
                t1 = s2.tile([P, TOWN], F32, tag="qt1")
                t2 = s2.tile([P, TOWN], F32, tag="qt2")
                nc.vector.tensor_mul(t1[:], ps_q0[:], cos_own[:])
                nc.vector.tensor_mul(t2[:], ps_q1[:], sin_own[:])
                nc.vector.tensor_sub(qTr[:, 2 * h], t1[:], t2[:])
                t3 = s2.tile([P, TOWN], F32, tag="qt3")
                t4 = s2.tile([P, TOWN], F32, tag="qt4")
                nc.vector.tensor_mul(t3[:], ps_q0[:], sin_own[:])
                nc.vector.tensor_mul(t4[:], ps_q1[:], cos_own[:])
                nc.vector.tensor_add(qTr[:, 2 * h + 1], t3[:], t4[:])

        # ============ P3: attention ============
        attn = p_attn.tile([P, 4, HID], BF16)  # [tok-part, qtile, head*D]
        with tc.tile_pool(name="s3", bufs=2) as s3, \
             tc.tile_pool(name="s3p", bufs=3) as s3p, \
             tc.tile_pool(name="ps3", bufs=2, space="PSUM") as ps3:
            m_sb = s3.tile([P, 4, TKV], BF16, tag="mask")
            nc.sync.dma_start(m_sb[:], d_mask[:])
            for qt in range(4):
                for h in range(NH):
                    sc = s3.tile([P, TKV], F32, tag="sc")
                    for c4 in range(TKV // 512):
                        ps_s = ps3.tile([P, 512], F32, tag="ps_s")
                        nc.tensor.matmul(ps_s[:], qTr[:, 2 * h, qt * P:(qt + 1) * P],
                                         kTr[:, 0, c4 * 512:(c4 + 1) * 512],
                                         start=True, stop=False)
                        nc.tensor.matmul(ps_s[:], qTr[:, 2 * h + 1, qt * P:(qt + 1) * P],
                                         kTr[:, 1, c4 * 512:(c4 + 1) * 512],
                                         start=False, stop=True)
                        nc.vector.scalar_tensor_tensor(
                            sc[:, c4 * 512:(c4 + 1) * 512], ps_s[:], SCALING,
                            m_sb[:, qt, c4 * 512:(c4 + 1) * 512], ALU.mult, ALU.add)
                    nm = s3.tile([P, 1], F32, tag="nm")
                    nc.vector.reduce_max(nm[:], sc[:], axis=mybir.AxisListType.X,
                                         negate=True)
                    pr = s3.tile([P, TKV], BF16, tag="pr")
                    se = s3.tile([P, 1], F32, tag="se")
                    nc.scalar.activation(pr[:], sc[:], AF.Exp, bias=nm[:], scale=1.0,
                                         accum_out=se[:])
                    rc = s3.tile([P, 1], F32, tag="rc")
                    nc.vector.reciprocal(rc[:], se[:])
                    ps_o = ps3.tile([P, D], F32, tag="ps_pv")
                    for kb in range(TKV // P):
                        ps_t = ps3.tile([P, P], BF16, tag="ps_t")
                        nc.tensor.transpose(ps_t[:], pr[:, kb * P:(kb + 1) * P],
                                            ident_b[:])
                        pT = s3p.tile([P, P], BF16, tag="pT")
                        nc.vector.tensor_copy(pT[:], ps_t[:])
                        nc.tensor.matmul(ps_o[:], pT[:], v_sb[:, kb],
                                         start=(kb == 0), stop=(kb == TKV // P - 1),
                                         skip_group_check=True)
                    nc.vector.tensor_scalar_mul(attn[:, qt, h * D:(h + 1) * D],
                                                ps_o[:], rc[:])
        p13_cm.__exit__(None, None, None)

        # ============ P4: transpose attn -> attnT [feat-part, tok] ============
        p_rest_cm = tc.tile_pool(name="p_rest", bufs=1, side="right")  # P4..end
        p_rest = p_rest_cm.__enter__()
        resT = p_rest.tile([P, KC, TOWN], F32)
        p_attnT_cm = tc.tile_pool(name="p_attnT", bufs=1, side="right")  # P4-P5
        p_attnT = p_attnT_cm.__enter__()
        attnT = p_attnT.tile([P, KC, TOWN], BF16)
        with tc.tile_pool(name="ps4", bufs=2, space="PSUM") as ps4:
            for c in range(KC):
                for qt in range(4):
                    ps_t2 = ps4.tile([P, P], BF16, tag="ps_t2")
                    nc.tensor.transpose(ps_t2[:], attn[:, qt, c * P:(c + 1) * P],
                                        ident_b[:])
                    nc.vector.tensor_copy(attnT[:, c, qt * P:(qt + 1) * P], ps_t2[:])
        p_attn_cm.__exit__(None, None, None)

        # ============ P5: O-projection + residual ============
        with tc.tile_pool(name="s5", bufs=2) as s5, \
             tc.tile_pool(name="ps5", bufs=2, space="PSUM") as ps5:
            for j in range(KC // 2):
                ow_t = s5.tile([P, KC, 2 * P], BF16, tag="ow")
                nc.sync.dma_start(ow_t[:], d_ow[:, j])
                for half in range(2):
                    ht = 2 * j + half
                    ps = ps5.tile([P, TOWN], F32, tag="ps_op")
                    for k in range(KC):
                        nc.tensor.matmul(ps[:], ow_t[:, k, half * P:(half + 1) * P],
                                         attnT[:, k], start=(k == 0), stop=(k == KC - 1))
                    hsres = s5.tile([P, TOWN], F32, tag="hsres")
                    nc.sync.dma_start(hsres[:], d_hsown[:, ht])
                    nc.vector.tensor_add(resT[:, ht], ps[:], hsres[:])
        p_attnT_cm.__exit__(None, None, None)

        # ============ P6: norm2 ============
        h2 = p_rest.tile([P, KC, TOWN], BF16)
        with tc.tile_pool(name="s6", bufs=2) as s6, \
             tc.tile_pool(name="ps6", bufs=2, space="PSUM") as ps6:
            rms_norm(s6, ps6, resT, h2, TOWN, "n2")

        # ============ P7: MLP (GeGLU) with 4 inter-blocks ============
        with tc.tile_pool(name="s7", bufs=2) as s7, \
             tc.tile_pool(name="s7w", bufs=4) as s7w, \
             tc.tile_pool(name="ps7", bufs=2, space="PSUM") as ps7:
            acc = p_rest.tile([P, KC, TOWN], F32)
            geglu = p_rest.tile([P, 32, TOWN], BF16)
            for blk in range(4):
                for jj in range(16):  # pairs of inter tiles within block
                    j = blk * 16 + jj
                    gw_t = s7w.tile([P, KC, 2 * P], BF16, tag="w8")
                    uw_t = s7w.tile([P, KC, 2 * P], BF16, tag="w8")
                    nc.sync.dma_start(gw_t[:], d_gw[:, j])
                    nc.sync.dma_start(uw_t[:], d_uw[:, j])
                    for half in range(2):
                        il = jj * 2 + half
                        ps_g = ps7.tile([P, TOWN], F32, tag="ps_g")
                        ps_u = ps7.tile([P, TOWN], F32, tag="ps_u")
                        for k in range(KC):
                            nc.tensor.matmul(ps_g[:], gw_t[:, k, half * P:(half + 1) * P],
                                             h2[:, k], start=(k == 0), stop=(k == KC - 1))
                        for k in range(KC):
                            nc.tensor.matmul(ps_u[:], uw_t[:, k, half * P:(half + 1) * P],
                                             h2[:, k], start=(k == 0), stop=(k == KC - 1))
                        gl = s7.tile([P, TOWN], F32, tag="gl")
                        nc.scalar.activation(gl[:], ps_g[:], AF.Gelu_apprx_tanh)
                        nc.vector.tensor_mul(geglu[:, il], gl[:], ps_u[:])
                for ht in range(KC):
                    dw_t = s7w.tile([P, 32, P], BF16, tag="dw")
                    nc.sync.dma_start(dw_t[:], d_dw[:, ht, blk * 32:(blk + 1) * 32])
                    ps_d = ps7.tile([P, TOWN], F32, tag="ps_d")
                    for kc in range(32):
                        nc.tensor.matmul(ps_d[:], dw_t[:, kc], geglu[:, kc],
                                         start=(kc == 0), stop=(kc == 31))
                    if blk == 0:
                        nc.vector.tensor_add(acc[:, ht], ps_d[:], resT[:, ht])
                    else:
                        nc.vector.tensor_add(acc[:, ht], acc[:, ht], ps_d[:])
            nc.sync.dma_start(d_out.rearrange("c p t -> p c t"), acc[:])
        p_rest_cm.__exit__(None, None, None)

    nc.compile()
    return nc


def _bf16(x):
    return np.asarray(x, dtype=np.float32).astype(ml_dtypes.bfloat16)


def _pack_pairs(w):
    """w [F, HID] -> [P, F//256, KC, 256]: out[p, j, k, i] = w[j*256+i, k*128+p]"""
    F = w.shape[0]
    return np.ascontiguousarray(
        w.reshape(F // 256, 256, KC, P).transpose(3, 0, 2, 1))


def _prep_in_maps(inputs):
    hidden = np.asarray(inputs["hidden_states"], np.float32)
    fc = np.asarray(inputs["freqs_cos"], np.float32)
    fs = np.asarray(inputs["freqs_sin"], np.float32)
    idx = np.asarray(inputs["kv_write_indices"])
    mask = np.asarray(inputs["mask"], np.float32)
    q_w = np.asarray(inputs["q_w"], np.float32)
    k_w = np.asarray(inputs["k_w"], np.float32)
    v_w = np.asarray(inputs["v_w"], np.float32)
    o_w = np.asarray(inputs["o_w"], np.float32)
    gate_w = np.asarray(inputs["gate_w"], np.float32)
    up_w = np.asarray(inputs["up_w"], np.float32)
    down_w = np.asarray(inputs["down_w"], np.float32)
    ln1 = np.asarray(inputs["ln1_w"], np.float32)
    ln2 = np.asarray(inputs["ln2_w"], np.float32)
    assert np.array_equal(idx.astype(np.int64), np.arange(S, dtype=np.int64)), \
        "kernel assumes kv_write_indices == arange(S)"

    s1 = 1.0 + ln1
    s2 = 1.0 + ln2
    qw = _bf16(_pack_pairs(q_w * s1[None, :]))
    kwp = _bf16((k_w * s1[None, :]).reshape(2, P, KC, P).transpose(3, 0, 2, 1))
    vwp = _bf16((v_w * s1[None, :]).reshape(D, KC, P).transpose(2, 1, 0))
    owp = _bf16(_pack_pairs(o_w))
    gwp = _bf16(_pack_pairs(gate_w * s2[None, :]))
    uwp = _bf16(_pack_pairs(up_w * s2[None, :]))
    dwp = _bf16(down_w.reshape(KC, P, INTER // P, P).transpose(3, 0, 2, 1))
    cosT = np.ascontiguousarray(fc.T)  # [128, 2048]
    sinT = np.ascontiguousarray(fs.T)

    shared = dict(qw=qw, kw=np.ascontiguousarray(kwp), vw=np.ascontiguousarray(vwp),
                  ow=owp, gw=gwp, uw=uwp, dw=np.ascontiguousarray(dwp),
                  coskv=_bf16(cosT), sinkv=_bf16(sinT))
    in_maps = []
    for c in range(8):
        b, t0 = c // 4, (c % 4) * TOWN
        hsT_b = hidden[b].T  # [HID, TKV]
        m = dict(shared)
        m["hskv"] = np.ascontiguousarray(
            hsT_b.reshape(KC, P, TKV).transpose(1, 0, 2))
        m["hsown"] = np.ascontiguousarray(
            hsT_b[:, t0:t0 + TOWN].reshape(KC, P, TOWN).transpose(1, 0, 2))
        m["cosown"] = _bf16(np.ascontiguousarray(cosT[:, t0:t0 + TOWN]))
        m["sinown"] = _bf16(np.ascontiguousarray(sinT[:, t0:t0 + TOWN]))
        m["mask"] = _bf16(
            mask[0, 0, t0:t0 + TOWN, :].reshape(4, P, TKV).transpose(1, 0, 2))
        in_maps.append(m)
    return in_maps


def kernel(**inputs):
    global _NC_CACHE, LAST_RESULTS
    in_maps = _prep_in_maps(inputs)
    if _NC_CACHE is None:
        _NC_CACHE = _build_nc()
    nc = _NC_CACHE
    res = run_bass_kernel_spmd(nc, in_maps, core_ids=list(range(8)), trace=TRACE)
    LAST_RESULTS = res

    out = np.empty((B, S, HID), np.float32)
    for c in range(8):
        b, t0 = c // 4, (c % 4) * TOWN
        o = res.results[c]["out"]  # [KC, P, TOWN]
        out[b, t0:t0 + TOWN, :] = o.reshape(HID, TOWN).T
    return out


# revision 13
# speedup vs baseline: 1.0275x; 1.0275x over previous
"""Gemma decoder layer on 8 Trainium2 NeuronCores.

Strategy: token-parallel (data-parallel over tokens), zero collectives.
Each core owns a 512-token strip (batch c//4, positions (c%4)*512..+512).
All weights replicated (bf16); every core computes full K/V for its batch
(MQA => tiny). All activations live in transposed layout [hidden->partitions,
tokens->free]; RMSNorm partition-reduction via ones-matmul; norm weights
(1+w) folded into the following matmul weights on the host. The program is
identical across cores (SPMD) - all per-core variation enters as input data.

Matmuls in bf16 (fp32 PSUM accumulation); softmax/norm/rope/residual in fp32.
"""

from contextlib import ExitStack

import numpy as np
import ml_dtypes

import concourse.bass as bass
import concourse.mybir as mybir
import concourse.tile as tile
from concourse import bacc
from concourse.bass_utils import run_bass_kernel_spmd
from concourse.masks import make_identity

P = 128
B, S, HID = 2, 2048, 2048
NH, D = 8, 256
INTER = 16384
KC = HID // P          # 16 hidden chunks
TOWN = 512             # own tokens per core
TKV = 2048             # tokens per batch (keys)
EPS = 1e-6
SCALING = D ** -0.5
KCAPS = [16, 12, 8, 4]  # causal keyblock caps per qtile slot (uniform across cores)
F32 = mybir.dt.float32
BF16 = mybir.dt.bfloat16
AF = mybir.ActivationFunctionType
ALU = mybir.AluOpType

TRACE = False
LAST_RESULTS = None
_NC_CACHE = None


def _build_nc():
    nc = bacc.Bacc(None, target_bir_lowering=False)

    # ---- DRAM I/O (shapes match SBUF layouts exactly; host does all packing) ----
    d_hskv = nc.dram_tensor("hskv", [P, KC, TKV], F32, kind="ExternalInput")
    d_hsown = nc.dram_tensor("hsown", [P, KC, TOWN], F32, kind="ExternalInput")
    d_coskv = nc.dram_tensor("coskv", [P, TKV], BF16, kind="ExternalInput")
    d_sinkv = nc.dram_tensor("sinkv", [P, TKV], BF16, kind="ExternalInput")
    d_cosown = nc.dram_tensor("cosown", [P, TOWN], BF16, kind="ExternalInput")
    d_sinown = nc.dram_tensor("sinown", [P, TOWN], BF16, kind="ExternalInput")
    d_mask = nc.dram_tensor("mask", [P, 4, TKV], BF16, kind="ExternalInput")
    d_qw = nc.dram_tensor("qw", [P, NH, KC, 2 * P], BF16, kind="ExternalInput")
    d_kw = nc.dram_tensor("kw", [P, 2, KC, P], BF16, kind="ExternalInput")
    d_vw = nc.dram_tensor("vw", [P, KC, D], BF16, kind="ExternalInput")
    d_ow = nc.dram_tensor("ow", [P, KC // 2, KC, 2 * P], BF16, kind="ExternalInput")
    d_gw = nc.dram_tensor("gw", [P, INTER // (2 * P), KC, 2 * P], BF16, kind="ExternalInput")
    d_uw = nc.dram_tensor("uw", [P, INTER // (2 * P), KC, 2 * P], BF16, kind="ExternalInput")
    d_dw = nc.dram_tensor("dw", [P, KC, INTER // P, P], BF16, kind="ExternalInput")
    d_out = nc.dram_tensor("out", [KC, P, TOWN], F32, kind="ExternalOutput")

    with tile.TileContext(nc) as tc, ExitStack() as top:
        const = top.enter_context(tc.tile_pool(name="const", bufs=1))
        ones_b = const.tile([P, P], BF16)
        nc.vector.memset(ones_b[:], 1.0)
        eps_sb = const.tile([P, 1], F32)
        nc.vector.memset(eps_sb[:], EPS)
        ident_b = const.tile([P, P], BF16)
        make_identity(nc, ident_b)
        cos_kv = const.tile([P, TKV], BF16)
        sin_kv = const.tile([P, TKV], BF16)
        cos_own = const.tile([P, TOWN], BF16)
        sin_own = const.tile([P, TOWN], BF16)
        nc.sync.dma_start(cos_kv[:], d_coskv[:])
        nc.sync.dma_start(sin_kv[:], d_sinkv[:])
        nc.sync.dma_start(cos_own[:], d_cosown[:])
        nc.sync.dma_start(sin_own[:], d_sinown[:])
        kw_sb = const.tile([P, 2, KC, P], BF16)
        vw_sb = const.tile([P, KC, D], BF16)
        nc.sync.dma_start(kw_sb[:], d_kw[:])
        nc.sync.dma_start(vw_sb[:], d_vw[:])

        def rms_norm(sb, ps, src, dst, T, tagp):
            """src [P, KC, T] f32 -> dst [P, KC, T] bf16 (no (1+w): folded in weights)"""
            ps_ss = ps.tile([P, T], F32, tag=f"ss{tagp}")
            for k in range(KC):
                x2 = sb.tile([P, T], BF16, tag=f"x2{tagp}")
                nc.scalar.activation(x2[:], src[:, k], AF.Square)
                nc.tensor.matmul(ps_ss[:], ones_b[:], x2[:],
                                 start=(k == 0), stop=(k == KC - 1))
            sd = sb.tile([P, T], F32, tag=f"sd{tagp}")
            nc.scalar.activation(sd[:], ps_ss[:], AF.Sqrt, scale=1.0 / HID,
                                 bias=eps_sb[:])
            rs = sb.tile([P, T], F32, tag=f"rs{tagp}")
            nc.vector.reciprocal(rs[:], sd[:])
            for k in range(KC):
                nc.vector.tensor_mul(dst[:, k], src[:, k], rs[:])

        # pools that span multiple phases (entered/exited manually, LIFO per side)
        p_attn_cm = tc.tile_pool(name="p_attn", bufs=1)  # P1..P4 (attn tile, used P3-P4)
        p_attn = p_attn_cm.__enter__()
        p13_cm = tc.tile_pool(name="p13", bufs=1)        # P1..P3: kTr, v_sb, qTr
        p13 = p13_cm.__enter__()
        kTr = p13.tile([P, 2, TKV], BF16)
        v_sb = p13.tile([P, TKV // P, D], BF16)

        # ============ P1: per-batch K/V (norm + proj + rope for all TKV tokens) ====
        ST = 256  # strip size
        with tc.tile_pool(name="s1", bufs=3) as s1, \
             tc.tile_pool(name="ps1", bufs=2, space="PSUM") as ps1:
            for s in range(TKV // ST):
                hsT = s1.tile([P, KC, ST], F32, tag="hsT")
                nc.sync.dma_start(hsT[:], d_hskv[:, :, s * ST:(s + 1) * ST])
                hb = s1.tile([P, KC, ST], BF16, tag="hb")
                rms_norm(s1, ps1, hsT, hb, ST, "kv")
                # K projection (2 feature tiles: x1, x2 halves of the single KV head)
                ps_k0 = ps1.tile([P, ST], F32, tag="pk0")
                ps_k1 = ps1.tile([P, ST], F32, tag="pk1")
                for k in range(KC):
                    nc.tensor.matmul(ps_k0[:], kw_sb[:, 0, k], hb[:, k],
                                     start=(k == 0), stop=(k == KC - 1))
                for k in range(KC):
                    nc.tensor.matmul(ps_k1[:], kw_sb[:, 1, k], hb[:, k],
                                     start=(k == 0), stop=(k == KC - 1))
                cs = cos_kv[:, s * ST:(s + 1) * ST]
                sn = sin_kv[:, s * ST:(s + 1) * ST]
                t1 = s1.tile([P, ST], F32, tag="rt1")
                t2 = s1.tile([P, ST], F32, tag="rt2")
                nc.vector.tensor_mul(t1[:], ps_k0[:], cs)
                nc.vector.tensor_mul(t2[:], ps_k1[:], sn)
                nc.vector.tensor_sub(kTr[:, 0, s * ST:(s + 1) * ST], t1[:], t2[:])
                t3 = s1.tile([P, ST], F32, tag="rt3")
                t4 = s1.tile([P, ST], F32, tag="rt4")
                nc.vector.tensor_mul(t3[:], ps_k0[:], sn)
                nc.vector.tensor_mul(t4[:], ps_k1[:], cs)
                nc.vector.tensor_add(kTr[:, 1, s * ST:(s + 1) * ST], t3[:], t4[:])
                # V projection, natural layout [token-part, D]
                for tt in range(ST // P):
                    ps_v = ps1.tile([P, D], F32, tag="pv")
                    for k in range(KC):
                        nc.tensor.matmul(ps_v[:], hb[:, k, tt * P:(tt + 1) * P],
                                         vw_sb[:, k], start=(k == 0), stop=(k == KC - 1))
                    nc.vector.tensor_copy(v_sb[:, s * (ST // P) + tt], ps_v[:])

        # ============ P2: own strip norm + Q proj + rope ============
        qTr = p13.tile([P, KC, TOWN], BF16)
        with tc.tile_pool(name="s2a", bufs=1) as s2a, \
             tc.tile_pool(name="s2", bufs=2) as s2, \
             tc.tile_pool(name="ps2", bufs=2, space="PSUM") as ps2, \
             tc.tile_pool(name="ps2b", bufs=3, space="PSUM") as ps2b:
            hs_own = s2a.tile([P, KC, TOWN], F32)
            nc.sync.dma_start(hs_own[:], d_hsown[:])
            hb_own = s2a.tile([P, KC, TOWN], BF16)
            rms_norm(s2, ps2, hs_own, hb_own, TOWN, "own")
            for h in range(NH):
                qw_t = s2.tile([P, KC, 2 * P], BF16, tag="qw")
                nc.sync.dma_start(qw_t[:], d_qw[:, h])
                ps_q0 = ps2b.tile([P, TOWN], F32, tag="pq0")
                ps_q1 = ps2b.tile([P, TOWN], F32, tag="pq1")
                for k in range(KC):
                    nc.tensor.matmul(ps_q0[:], qw_t[:, k, 0:P], hb_own[:, k],
                                     start=(k == 0), stop=(k == KC - 1))
                for k in range(KC):
                    nc.tensor.matmul(ps_q1[:], qw_t[:, k, P:2 * P], hb_own[:, k],
                                     start=(k == 0), stop=(k == KC - 1))
                t1 = s2.tile([P, TOWN], F32, tag="qt1")
                t2 = s2.tile([P, TOWN], F32, tag="qt2")
                nc.vector.tensor_mul(t1[:], ps_q0[:], cos_own[:])
                nc.vector.tensor_mul(t2[:], ps_q1[:], sin_own[:])
                nc.vector.tensor_sub(qTr[:, 2 * h], t1[:], t2[:])
                t3 = s2.tile([P, TOWN], F32, tag="qt3")
                t4 = s2.tile([P, TOWN], F32, tag="qt4")
                nc.vector.tensor_mul(t3[:], ps_q0[:], sin_own[:])
                nc.vector.tensor_mul(t4[:], ps_q1[:], cos_own[:])
                nc.vector.tensor_add(qTr[:, 2 * h + 1], t3[:], t4[:])

        # ============ P3: attention ============
        attn = p_attn.tile([P, 4, HID], BF16)  # [tok-part, qtile, head*D]
        m_sb = p_attn.tile([P, 4, TKV], BF16)
        with tc.tile_pool(name="s3", bufs=3) as s3, \
             tc.tile_pool(name="s3p", bufs=4) as s3p, \
             tc.tile_pool(name="ps3a", bufs=3, space="PSUM") as ps3a, \
             tc.tile_pool(name="ps3b", bufs=3, space="PSUM") as ps3b:
            nc.sync.dma_start(m_sb[:], d_mask[:])
            for qt in range(4):
                KB = KCAPS[qt]          # keyblocks processed for this qtile slot
                KEYS = KB * P
                for h in range(NH):
                    sc = s3.tile([P, KEYS], F32, tag="sc")
                    for c4 in range(KEYS // 512):
                        ps_s = ps3a.tile([P, 512], F32, tag="ps_s")
                        nc.tensor.matmul(ps_s[:], qTr[:, 2 * h, qt * P:(qt + 1) * P],
                                         kTr[:, 0, c4 * 512:(c4 + 1) * 512],
                                         start=True, stop=False)
                        nc.tensor.matmul(ps_s[:], qTr[:, 2 * h + 1, qt * P:(qt + 1) * P],
                                         kTr[:, 1, c4 * 512:(c4 + 1) * 512],
                                         start=False, stop=True)
                        nc.vector.scalar_tensor_tensor(
                            sc[:, c4 * 512:(c4 + 1) * 512], ps_s[:], SCALING,
                            m_sb[:, qt, c4 * 512:(c4 + 1) * 512], ALU.mult, ALU.add)
                    nm = s3.tile([P, 1], F32, tag="nm")
                    nc.vector.reduce_max(nm[:], sc[:], axis=mybir.AxisListType.X,
                                         negate=True)
                    pr = s3.tile([P, KEYS], BF16, tag="pr")
                    se = s3.tile([P, 1], F32, tag="se")
                    nc.scalar.activation(pr[:], sc[:], AF.Exp, bias=nm[:], scale=1.0,
                                         accum_out=se[:])
                    rc = s3.tile([P, 1], F32, tag="rc")
                    nc.vector.reciprocal(rc[:], se[:])
                    pT_all = s3p.tile([P, KB, P], BF16, tag="pT")
                    nc.sync.dma_start_transpose(pT_all[:], pr[:])
                    ps_o = ps3b.tile([P, D], F32, tag="ps_pv")
                    for kb in range(KB):
                        nc.tensor.matmul(ps_o[:], pT_all[:, kb], v_sb[:, kb],
                                         start=(kb == 0), stop=(kb == KB - 1))
                    nc.vector.tensor_scalar_mul(attn[:, qt, h * D:(h + 1) * D],
                                                ps_o[:], rc[:])
        p13_cm.__exit__(None, None, None)

        # ============ P4: transpose attn -> attnT [feat-part, tok] ============
        p_rest_cm = tc.tile_pool(name="p_rest", bufs=1, side="right")  # P4..end
        p_rest = p_rest_cm.__enter__()
        resT = p_rest.tile([P, KC, TOWN], F32)
        p_attnT_cm = tc.tile_pool(name="p_attnT", bufs=1, side="right")  # P4-P5
        p_attnT = p_attnT_cm.__enter__()
        attnT = p_attnT.tile([P, KC, TOWN], BF16)
        for qt in range(4):
            nc.sync.dma_start_transpose(attnT[:, :, qt * P:(qt + 1) * P], attn[:, qt])
        p_attn_cm.__exit__(None, None, None)

        # ============ P5: O-projection + residual ============
        with tc.tile_pool(name="s5", bufs=2) as s5, \
             tc.tile_pool(name="ps5", bufs=2, space="PSUM") as ps5:
            for j in range(KC // 2):
                ow_t = s5.tile([P, KC, 2 * P], BF16, tag="ow")
                nc.sync.dma_start(ow_t[:], d_ow[:, j])
                for half in range(2):
                    ht = 2 * j + half
                    ps = ps5.tile([P, TOWN], F32, tag="ps_op")
                    for k in range(KC):
                        nc.tensor.matmul(ps[:], ow_t[:, k, half * P:(half + 1) * P],
                                         attnT[:, k], start=(k == 0), stop=(k == KC - 1))
                    hsres = s5.tile([P, TOWN], F32, tag="hsres")
                    nc.sync.dma_start(hsres[:], d_hsown[:, ht])
                    nc.vector.tensor_add(resT[:, ht], ps[:], hsres[:])
        p_attnT_cm.__exit__(None, None, None)

        # ============ P6: norm2 ============
        h2 = p_rest.tile([P, KC, TOWN], BF16)
        with tc.tile_pool(name="s6", bufs=2) as s6, \
             tc.tile_pool(name="ps6", bufs=2, space="PSUM") as ps6:
            rms_norm(s6, ps6, resT, h2, TOWN, "n2")

        # ============ P7: MLP (GeGLU) with 4 inter-blocks ============
        with tc.tile_pool(name="s7", bufs=2) as s7, \
             tc.tile_pool(name="s7w", bufs=4) as s7w, \
             tc.tile_pool(name="ps7", bufs=2, space="PSUM") as ps7:
            acc = p_rest.tile([P, KC, TOWN], F32)
            geglu = p_rest.tile([P, 32, TOWN], BF16)
            for blk in range(4):
                for jj in range(16):  # pairs of inter tiles within block
                    j = blk * 16 + jj
                    gw_t = s7w.tile([P, KC, 2 * P], BF16, tag="w8")
                    uw_t = s7w.tile([P, KC, 2 * P], BF16, tag="w8")
                    nc.sync.dma_start(gw_t[:], d_gw[:, j])
                    nc.sync.dma_start(uw_t[:], d_uw[:, j])
                    for half in range(2):
                        il = jj * 2 + half
                        ps_g = ps7.tile([P, TOWN], F32, tag="ps_g")
                        ps_u = ps7.tile([P, TOWN], F32, tag="ps_u")
                        for k in range(KC):
                            nc.tensor.matmul(ps_g[:], gw_t[:, k, half * P:(half + 1) * P],
                                             h2[:, k], start=(k == 0), stop=(k == KC - 1))
                        for k in range(KC):
                            nc.tensor.matmul(ps_u[:], uw_t[:, k, half * P:(half + 1) * P],
                                             h2[:, k], start=(k == 0), stop=(k == KC - 1))
                        gl = s7.tile([P, TOWN], F32, tag="gl")
                        nc.scalar.activation(gl[:], ps_g[:], AF.Gelu_apprx_tanh)
                        nc.vector.tensor_mul(geglu[:, il], gl[:], ps_u[:])
                for ht in range(KC):
                    dw_t = s7w.tile([P, 32, P], BF16, tag="dw")
                    nc.sync.dma_start(dw_t[:], d_dw[:, ht, blk * 32:(blk + 1) * 32])
                    ps_d = ps7.tile([P, TOWN], F32, tag="ps_d")
                    for kc in range(32):
                        nc.tensor.matmul(ps_d[:], dw_t[:, kc], geglu[:, kc],
                                         start=(kc == 0), stop=(kc == 31))
                    if blk == 0:
                        nc.vector.tensor_add(acc[:, ht], ps_d[:], resT[:, ht])
                    else:
                        nc.vector.tensor_add(acc[:, ht], acc[:, ht], ps_d[:])
                    if blk == 3:
                        nc.sync.dma_start(d_out[ht], acc[:, ht])
        p_rest_cm.__exit__(None, None, None)

    nc.compile()
    return nc


def _bf16(x):
    return np.asarray(x, dtype=np.float32).astype(ml_dtypes.bfloat16)


def _pack_pairs(w):
    """w [F, HID] -> [P, F//256, KC, 256]: out[p, j, k, i] = w[j*256+i, k*128+p]"""
    F = w.shape[0]
    return np.ascontiguousarray(
        w.reshape(F // 256, 256, KC, P).transpose(3, 0, 2, 1))


def _prep_in_maps(inputs):
    hidden = np.asarray(inputs["hidden_states"], np.float32)
    fc = np.asarray(inputs["freqs_cos"], np.float32)
    fs = np.asarray(inputs["freqs_sin"], np.float32)
    idx = np.asarray(inputs["kv_write_indices"])
    mask = np.asarray(inputs["mask"], np.float32)
    q_w = np.asarray(inputs["q_w"], np.float32)
    k_w = np.asarray(inputs["k_w"], np.float32)
    v_w = np.asarray(inputs["v_w"], np.float32)
    o_w = np.asarray(inputs["o_w"], np.float32)
    gate_w = np.asarray(inputs["gate_w"], np.float32)
    up_w = np.asarray(inputs["up_w"], np.float32)
    down_w = np.asarray(inputs["down_w"], np.float32)
    ln1 = np.asarray(inputs["ln1_w"], np.float32)
    ln2 = np.asarray(inputs["ln2_w"], np.float32)
    assert np.array_equal(idx.astype(np.int64), np.arange(S, dtype=np.int64)), \
        "kernel assumes kv_write_indices == arange(S)"

    s1 = 1.0 + ln1
    s2 = 1.0 + ln2
    qw = _bf16(_pack_pairs(q_w * s1[None, :]))
    kwp = _bf16((k_w * s1[None, :]).reshape(2, P, KC, P).transpose(3, 0, 2, 1))
    vwp = _bf16((v_w * s1[None, :]).reshape(D, KC, P).transpose(2, 1, 0))
    owp = _bf16(_pack_pairs(o_w))
    gwp = _bf16(_pack_pairs(gate_w * s2[None, :]))
    uwp = _bf16(_pack_pairs(up_w * s2[None, :]))
    dwp = _bf16(down_w.reshape(KC, P, INTER // P, P).transpose(3, 0, 2, 1))
    cosT = np.ascontiguousarray(fc.T)  # [128, 2048]
    sinT = np.ascontiguousarray(fs.T)

    shared = dict(qw=qw, kw=np.ascontiguousarray(kwp), vw=np.ascontiguousarray(vwp),
                  ow=owp, gw=gwp, uw=uwp, dw=np.ascontiguousarray(dwp),
                  coskv=_bf16(cosT), sinkv=_bf16(sinT))
    in_maps = []
    for c in range(8):
        b, cc = c // 4, c % 4
        jlist = _core_qtiles(cc)
        cols = np.concatenate([np.arange(j * P, (j + 1) * P) for j in jlist])
        hsT_b = hidden[b].T  # [HID, TKV]
        m = dict(shared)
        m["hskv"] = np.ascontiguousarray(
            hsT_b.reshape(KC, P, TKV).transpose(1, 0, 2))
        m["hsown"] = np.ascontiguousarray(
            hsT_b[:, cols].reshape(KC, P, TOWN).transpose(1, 0, 2))
        m["cosown"] = _bf16(np.ascontiguousarray(cosT[:, cols]))
        m["sinown"] = _bf16(np.ascontiguousarray(sinT[:, cols]))
        m["mask"] = _bf16(
            mask[0, 0, cols, :].reshape(4, P, TKV).transpose(1, 0, 2))
        in_maps.append(m)
    return in_maps


def _core_qtiles(cc):
    """Query-tile (128-token block) indices for core slot cc (0..3) within its
    batch: one block from each causal band so per-core processed key work is
    uniform under KCAPS."""
    return [12 + cc, 8 + cc, 4 + cc, cc]


def kernel(**inputs):
    global _NC_CACHE, LAST_RESULTS
    in_maps = _prep_in_maps(inputs)
    if _NC_CACHE is None:
        _NC_CACHE = _build_nc()
    nc = _NC_CACHE
    res = run_bass_kernel_spmd(nc, in_maps, core_ids=list(range(8)), trace=TRACE)
    LAST_RESULTS = res

    out = np.empty((B, S, HID), np.float32)
    for c in range(8):
        b, cc = c // 4, c % 4
        jlist = _core_qtiles(cc)
        o = res.results[c]["out"].reshape(HID, TOWN).T  # [TOWN, HID]
        for i, j in enumerate(jlist):
            out[b, j * P:(j + 1) * P, :] = o[i * P:(i + 1) * P, :]
    return out
